# revision 26
# baseline (speedup 1.0000x reference)
"""Distributed Trainium2 Bass kernel for a single attention head.

Problem (hardcoded): q,k,v [4, 4096, 1024] f32, Wq/Wk/Wv [1024, 64] f32,
attn_mask [4096, 4096] bool (True = keep).  out[b] = softmax(mask(q Wq (k Wk)^T) / 8) (v Wv).

Sharding: 8 cores; core c -> batch c//2, and query chunks {2s + c%2 : s in 0..3}
(512 rows each, interleaved so the causal-mask work per program slot is uniform
across cores -- all cores execute one SPMD program).

v7: bf16 operands (host casts).  PE is kept gapless so it stays at its
2.4 GHz p-state: causal mask applied on the PE (Atri^T @ B accumulated into
the score PSUM, B one-hot at first dropped k), chunk ch+1's projections
drained as filler between chunk ch's attention matmuls, PV pipelined one
group behind scores.  Attention work items (k-tile, slot) are packed two
per [128,1024] score PSUM in flat order -- pairs span t where only one slot
is live -- so one exp instruction always covers two items (halves the
activation-bound tail).  Host pre-arranges q/k/v/w/mask into the exact SBUF
layouts (chunk-major [128, dt, cols]) so every DMA is one contiguous run
per partition (cheap descriptor generation); input DMAs issue from the
otherwise-idle gpsimd queue.  Slot 3's first extents[0] k-tiles are
deferred to the tail so slots 0 and 3 share a PSUM bank, freeing a
dedicated projection-PSUM bank.
"""

import os
import sys

sys.path.insert(0, "/opt/trn_rl_repo")

import numpy as np
import ml_dtypes

import concourse.bass as bass
import concourse.mybir as mybir
import concourse.tile as tile
from concourse import bacc
from concourse.bass_utils import run_bass_kernel_spmd
from concourse.masks import make_identity

F32 = mybir.dt.float32
BF16 = mybir.dt.bfloat16
BF = ml_dtypes.bfloat16

N_CORES = 8
B, T, D, H = 4, 4096, 1024, 64
P = 128                      # partitions
QC = 512                     # query chunk width
N_CHUNKS = T // QC           # 8 global query chunks
N_SLOTS = N_CHUNKS // 2      # 4 chunks per core
KT = T // P                  # 32 k-tiles of 128 rows
D_TILES = D // P             # 8
XCW = 1024                   # x-chunk width for streaming projections
TQ = N_SLOTS * QC            # 2048 local query rows per core
KPC = XCW // P               # k-tiles per chunk
NQCH = TQ // XCW             # q chunks
NKCH = T // XCW              # k/v chunks (full extent)
NEG = -1.0e30                # additive mask value for dropped positions

LAST_RESULT = None           # test harness reads exec_time_ns from here
_CACHE = {}


def _mask_schedule(mask):
    """Derive the compile-time schedule from the actual mask.

    Returns (extents, window, mode):
      mode 'mm': window blocks are k-suffix drops, content slot-independent
                 per parity -> mask as a PE matmul (one-hot B tiles).
      mode 'add': consistent but not suffix -> resident additive DVE tiles.
      mode 'stream': general fallback, tiles streamed from HBM per (s,t).
    """
    m = mask.reshape(N_CHUNKS, QC, KT, P)
    blk_any = m.any(axis=(1, 3))   # [chunk, ktile]
    blk_all = m.all(axis=(1, 3))
    extents = []
    window = []
    for s in range(N_SLOTS):
        js = (2 * s, 2 * s + 1)
        ext = 1
        for j in js:
            nz = np.nonzero(blk_any[j])[0]
            if len(nz):
                ext = max(ext, int(nz[-1]) + 1)
        extents.append(ext)
        window.append(tuple(
            t for t in range(ext) if (~blk_all[js, t]).any()))
    nj = max((len(w) for w in window), default=0)
    consistent = nj <= 16
    suffix = True
    if consistent:
        for par in range(2):
            for j in range(nj):
                blocks = [
                    m[2 * s + par, :, window[s][j], :]
                    for s in range(N_SLOTS) if j < len(window[s])
                ]
                if any(not np.array_equal(blocks[0], bb) for bb in blocks[1:]):
                    consistent = False
                drop = ~blocks[0]            # [qr, kc]
                if not (drop[:, :-1] <= drop[:, 1:]).all():
                    suffix = False
    mode = ('mm' if consistent and suffix else
            'add' if consistent else 'stream')
    return tuple(extents), tuple(window), mode


def _build(extents, window, mode):
    nj = max((len(w) for w in window), default=0)
    n_mask = max(1, sum(len(w) for w in window))   # stream mode count
    mdt = BF16 if mode == 'mm' else F32
    nc = bacc.Bacc("TRN2", target_bir_lowering=False, debug=False,
                   num_devices=N_CORES)
    qT = nc.dram_tensor("qT", [NQCH, P, D_TILES, XCW], BF16,
                        kind="ExternalInput")
    kT = nc.dram_tensor("kT", [NKCH, P, D_TILES, XCW], BF16,
                        kind="ExternalInput")
    vT = nc.dram_tensor("vT", [NKCH, P, D_TILES, XCW], BF16,
                        kind="ExternalInput")
    w = nc.dram_tensor("w", [P, D_TILES, 3 * H], BF16, kind="ExternalInput")
    if mode == 'stream':
        maskp = nc.dram_tensor("maskp", [n_mask, P, QC], F32,
                               kind="ExternalInput")
    else:
        maskp = nc.dram_tensor("maskp", [P, max(1, nj), QC], mdt,
                               kind="ExternalInput")
    atri = nc.dram_tensor("atri", [P, P], BF16, kind="ExternalInput")
    out = nc.dram_tensor("out", [TQ, H], F32, kind="ExternalOutput")

    Exp = mybir.ActivationFunctionType.Exp
    kt_lim = max(extents)
    n_kv_chunks = (kt_lim * P + XCW - 1) // XCW
    jmap = {}
    for s in range(N_SLOTS):
        for j, t in enumerate(window[s]):
            jmap[(s, t)] = j
    stream_order = {}
    for t in range(kt_lim):
        for s in range(N_SLOTS):
            if t < extents[s] and (s, t) in jmap:
                stream_order[(s, t)] = len(stream_order)

    # slot-3 deferral: share one PSUM bank between slot 0 and slot 3 by
    # processing slot 3's first extents[0] k-tiles after everything else.
    DEF = N_SLOTS - 1
    E0 = extents[0]
    defer = extents[DEF] > E0
    def live_at(t, s):
        return t < extents[s] and not (defer and s == DEF and t < E0)

    items = []
    # chunk 0: slot-pair-major so early groups need only the first half of
    # q (slots 0,1) while q's second half is still loading
    for spair in range(2):
        for t in range(min(KPC, kt_lim)):
            for s in (2 * spair, 2 * spair + 1):
                if live_at(t, s):
                    items.append((t, s))
    for t in range(KPC, kt_lim):
        for s in range(N_SLOTS):
            if live_at(t, s):
                items.append((t, s))
    if defer:
        # zip the deferred slot-3 items into the final stretch (which is
        # mostly solo slot-3 k-tiles) so tail groups stay 2 items wide
        defi = [(t, DEF) for t in range(E0)]
        kzip = min(len(defi), len(items))
        tail = items[len(items) - kzip:]
        items = items[:len(items) - kzip]
        for a, b in zip(tail, defi):
            items.extend((a, b))
        items.extend(defi[kzip:])
    groups = [items[i:i + 2] for i in range(0, len(items), 2)]
    first_t = {}
    last_t = {}
    for t, s in items:
        first_t.setdefault(s, t)
        last_t[s] = t

    with tile.TileContext(nc) as tc:
        with (
            tc.tile_pool(name="const", bufs=1) as cpool,
            tc.tile_pool(name="qkh", bufs=1) as qkhpool,
            tc.tile_pool(name="vh", bufs=1) as vhpool,
            tc.tile_pool(name="oacc", bufs=1, space="PSUM") as opool,
        ):
            w_sb = cpool.tile([P, D_TILES, 3 * H], BF16)
            nc.sync.dma_start(out=w_sb[:], in_=w.ap())
            ident_b = cpool.tile([P, P], BF16)
            make_identity(nc, ident_b[:])
            ident_f = cpool.tile([H + 1, H + 1], F32)
            make_identity(nc, ident_f[:])
            atri_sb = None
            maskp_sb = None
            if mode == 'mm':
                atri_sb = cpool.tile([P, P], BF16)
                nc.sync.dma_start(out=atri_sb[:], in_=atri.ap())
            if mode != 'stream' and nj:
                maskp_sb = cpool.tile([P, nj, QC], mdt)
                # (its DMA is issued in the head sequence, after q0/k0)

            qhT = qkhpool.tile([H, TQ], BF16, tag="qhT")
            khT = qkhpool.tile([H, T], BF16, tag="khT")
            vh1 = vhpool.tile([P, KT, H + 1], BF16)

            oaccs = {}

            def oacc_alloc(s):
                tag = "oaccA" if (s in (0, DEF) and defer) else f"oacc{s}"
                oaccs[s] = opool.tile([H + 1, QC], F32, tag=tag,
                                      name=f"oacc{s}")
            for s in range(N_SLOTS):
                if not (defer and s == DEF):
                    oacc_alloc(s)

            with (
                tc.tile_pool(name="xs", bufs=4) as xpool,
                tc.tile_pool(name="ps", bufs=2, space="PSUM") as spool,
                tc.tile_pool(name="pps", bufs=1, space="PSUM") as pppool,
                tc.tile_pool(name="pt", bufs=6) as ppool,
                tc.tile_pool(name="vsb", bufs=2) as vsbpool,
                tc.tile_pool(name="mt", bufs=3) as mpool,
                tc.tile_pool(name="osb", bufs=2) as osbpool,
                tc.tile_pool(name="rec", bufs=2) as recpool,
                tc.tile_pool(name="ob", bufs=2) as obpool,
            ):
                proj_pool = pppool if defer else spool
                ppbig = None
                if defer:
                    # persistent [128, QC] PSUM bank: projection chains use
                    # alternating 64-partition halves (no WAR between
                    # consecutive chains); transposes/epilogue use [:, 0:65].
                    ppbig = pppool.tile([P, QC], F32, tag="pp", name="ppbig")
                chain_par = [0]

                def proj_psum():
                    if ppbig is None:
                        return proj_pool.tile([H, QC], F32, tag="pp")
                    r0 = chain_par[0] * H
                    chain_par[0] ^= 1
                    return ppbig[r0:r0 + H, :]

                def tp_psum(dtype):
                    if ppbig is None:
                        return proj_pool.tile([P, H + 1], dtype,
                                              tag="pp", name="tp")[:]
                    if dtype == F32:
                        return ppbig[:, 0:H + 1]
                    return ppbig[:].bitcast(dtype)[:, 0:H + 1]

                def proj_chain(dst, dst_col, src_sb, n, wlo, whi):
                    """8-matmul chain projecting src cols [n*QC,(n+1)*QC);
                    DVE-copies the psum into dst[0:H, dst_col:+QC]."""
                    pp = proj_psum()
                    for dt_ in range(D_TILES):
                        nc.tensor.matmul(
                            pp,
                            lhsT=w_sb[:, dt_, wlo:whi],
                            rhs=src_sb[:, dt_, n * QC:(n + 1) * QC],
                            start=(dt_ == 0), stop=(dt_ == D_TILES - 1))
                        if dt_ % 4 == 3:
                            yield
                    nc.vector.tensor_scalar_mul(
                        dst[0:H, dst_col:dst_col + QC], pp, 1.0)
                    yield

                def kv_gen(ch):
                    """chunk ch's k/v projection, yielded in filler units."""
                    base = ch * XCW
                    ktx, vtx = xts[ch]
                    for n in range(XCW // QC):
                        yield from proj_chain(khT, base + n * QC, ktx, n,
                                              H, 2 * H)
                    vsb = vsbpool.tile([H + 1, XCW], BF16, tag="vsb")
                    nc.vector.memset(vsb[H:H + 1, :], 1.0)
                    for n in range(XCW // QC):
                        yield from proj_chain(vsb, n * QC, vtx, n,
                                              2 * H, 3 * H)
                    for kt_ in range(KPC):
                        t_glob = ch * KPC + kt_
                        if t_glob >= kt_lim:
                            continue
                        tp = tp_psum(BF16)
                        nc.tensor.transpose(
                            tp, vsb[:, kt_ * P:(kt_ + 1) * P],
                            ident_b[0:H + 1, 0:H + 1])
                        nc.vector.tensor_scalar_mul(
                            vh1[:, t_glob, :], tp, 1.0)
                        if kt_ % 2 == 1:
                            yield

                def dma_k(ch):
                    ktx = xpool.tile([P, D_TILES, XCW], BF16, tag="x")
                    nc.sync.dma_start(out=ktx[:], in_=kT[ch])
                    return ktx

                def dma_v(ch):
                    vtx = xpool.tile([P, D_TILES, XCW], BF16, tag="x")
                    nc.sync.dma_start(out=vtx[:], in_=vT[ch])
                    return vtx

                def dma_chunk(ch):
                    xts[ch] = (dma_k(ch), dma_v(ch))

                def emit_group(group):
                    """scores (+mask) + one exp for <=2 (t, s) items."""
                    wt = spool.tile([P, 2 * QC], F32, tag="ps")
                    pt = ppool.tile([P, 2 * QC], BF16, tag="p")
                    for idx, (t, s) in enumerate(group):
                        ho = idx * QC
                        mm_mask = mode == 'mm' and (s, t) in jmap
                        nc.tensor.matmul(
                            wt[:, ho:ho + QC],
                            lhsT=khT[:, t * P:(t + 1) * P],
                            rhs=qhT[:, s * QC:(s + 1) * QC],
                            start=True, stop=not mm_mask)
                        if mm_mask:
                            nc.tensor.matmul(
                                wt[:, ho:ho + QC],
                                lhsT=atri_sb[:],
                                rhs=maskp_sb[:, jmap[(s, t)], :],
                                start=False, stop=True)
                        elif (s, t) in jmap:
                            if mode == 'add':
                                nc.vector.tensor_add(
                                    wt[:, ho:ho + QC], wt[:, ho:ho + QC],
                                    maskp_sb[:, jmap[(s, t)], :])
                            else:
                                m = mpool.tile([P, QC], F32, tag="m")
                                nc.sync.dma_start(
                                    out=m[:],
                                    in_=maskp[stream_order[(s, t)]])
                                nc.vector.tensor_add(
                                    wt[:, ho:ho + QC], wt[:, ho:ho + QC],
                                    m[:])
                    hi = len(group) * QC
                    nc.scalar.activation(
                        out=pt[:, 0:hi], in_=wt[:, 0:hi],
                        func=Exp, scale=0.125)
                    return [(t, s, pt, idx * QC)
                            for idx, (t, s) in enumerate(group)]

                def emit_pv(pv_items):
                    for (t, s, pt, ho) in pv_items:
                        if s not in oaccs:
                            oacc_alloc(s)
                        nc.tensor.matmul(
                            oaccs[s][:],
                            lhsT=vh1[:, t, :],
                            rhs=pt[:, ho:ho + QC],
                            start=(t == first_t[s]), stop=(t == last_t[s]))
                        if t == last_t[s]:
                            epilogue(s)

                def epilogue(s):
                    osb = osbpool.tile([H + 1, QC], F32, tag="osb")
                    nc.vector.tensor_scalar_mul(osb[:], oaccs[s][:], 1.0)
                    for jj in range(QC // P):
                        ot = tp_psum(F32)
                        nc.tensor.transpose(
                            ot, osb[:, jj * P:(jj + 1) * P], ident_f[:])
                        rec = recpool.tile([P, 1], F32, tag="rec")
                        nc.vector.reciprocal(rec[:], ot[:, H:H + 1])
                        ob = obpool.tile([P, H], F32, tag="ob")
                        nc.vector.tensor_scalar_mul(ob[:], ot[:, 0:H], rec[:])
                        r0 = s * QC + jj * P
                        nc.sync.dma_start(out=out[r0:r0 + P, :], in_=ob[:])

                xts = {}

                # head: issue DMAs in consumption order and project each
                # tensor as it arrives: q0, k0, masks, q1, v0
                q0x = xpool.tile([P, D_TILES, XCW], BF16, tag="x")
                nc.sync.dma_start(out=q0x[:, :, 0:QC], in_=qT[0][:, :, 0:QC])
                nc.sync.dma_start(out=q0x[:, :, QC:XCW],
                                  in_=qT[0][:, :, QC:XCW])
                ktx0 = xpool.tile([P, D_TILES, XCW], BF16, tag="x")
                nc.sync.dma_start(out=ktx0[:, :, 0:QC], in_=kT[0][:, :, 0:QC])
                nc.sync.dma_start(out=ktx0[:, :, QC:XCW],
                                  in_=kT[0][:, :, QC:XCW])
                if maskp_sb is not None:
                    nc.sync.dma_start(out=maskp_sb[:], in_=maskp.ap())
                q1x = None
                if NQCH > 1:
                    q1x = xpool.tile([P, D_TILES, XCW], BF16, tag="x")
                    nc.sync.dma_start(out=q1x[:], in_=qT[1])
                vtx0 = dma_v(0)
                xts[0] = (ktx0, vtx0)
                for n in range(XCW // QC):
                    for _ in proj_chain(qhT, n * QC, q0x, n, 0, H):
                        pass
                for n in range(XCW // QC):
                    for _ in proj_chain(khT, n * QC, ktx0, n, H, 2 * H):
                        pass
                if q1x is not None:
                    for n in range(XCW // QC):
                        for _ in proj_chain(qhT, XCW + n * QC, q1x, n, 0, H):
                            pass
                vsb0 = vsbpool.tile([H + 1, XCW], BF16, tag="vsb")
                nc.vector.memset(vsb0[H:H + 1, :], 1.0)
                for n in range(XCW // QC):
                    for _ in proj_chain(vsb0, n * QC, vtx0, n, 2 * H, 3 * H):
                        pass
                for kt_ in range(min(KPC, kt_lim)):
                    tp = tp_psum(BF16)
                    nc.tensor.transpose(
                        tp, vsb0[:, kt_ * P:(kt_ + 1) * P],
                        ident_b[0:H + 1, 0:H + 1])
                    nc.vector.tensor_scalar_mul(vh1[:, kt_, :], tp, 1.0)

                # flat group loop: drain chunk ch+1's projection as filler
                prev = None
                gen = None
                gen_ch = 0
                dma_hi = 0
                seen_ch = -1
                for group in groups:
                    ch = max(t for t, _ in group) // KPC
                    if ch > seen_ch:
                        seen_ch = ch
                        if gen is not None:
                            for _ in gen:
                                pass
                            gen = None
                        for c in (ch + 1, ch + 2):
                            if c < n_kv_chunks and c > dma_hi:
                                dma_chunk(c)
                                dma_hi = c
                        if ch + 1 < n_kv_chunks and ch + 1 > gen_ch:
                            gen = kv_gen(ch + 1)
                            gen_ch = ch + 1
                    cur = emit_group(group)
                    if gen is not None:
                        for _ in range(3):
                            if next(gen, 'DONE') == 'DONE':
                                gen = None
                                break
                    if prev is not None:
                        emit_pv(prev)
                    prev = cur
                if gen is not None:
                    for _ in gen:
                        pass
                if prev is not None:
                    emit_pv(prev)

    nc.compile()
    return nc


def _get_nc(extents, window, mode):
    key = (extents, window, mode)
    if key not in _CACHE:
        _CACHE[key] = _build(extents, window, mode)
    return _CACHE[key]


def _atri():
    i = np.arange(P)
    return (np.float32(NEG) * (i[:, None] <= i[None, :])).astype(BF)


def _chunk_major(xT):
    """[D, C] (d-major) -> [C//XCW, 128, 8, XCW] so each DMA chunk is one
    contiguous run per partition."""
    ncol = xT.shape[1]
    return np.ascontiguousarray(
        xT.reshape(D_TILES, P, ncol // XCW, XCW).transpose(2, 1, 0, 3))


def _make_in_maps(q, k, v, Wq, Wk, Wv, mask, extents, window, mode):
    nj = max((len(w) for w in window), default=0)
    wcat = np.concatenate(
        [np.asarray(Wq), np.asarray(Wk), np.asarray(Wv)], axis=1).astype(BF)
    w2 = np.ascontiguousarray(
        wcat.reshape(D_TILES, P, 3 * H).transpose(1, 0, 2))
    kTb = [_chunk_major(k[b].T.astype(BF)) for b in range(B)]
    vTb = [_chunk_major(v[b].T.astype(BF)) for b in range(B)]
    qTb = [np.ascontiguousarray(q[b].T.astype(BF)) for b in range(B)]
    mm = mask.reshape(N_CHUNKS, QC, KT, P)
    atri = _atri()

    def add_tile(g, t):
        # [128 k, 512 q] additive f32 tile for (chunk g, k-tile t)
        return np.where(mm[g, :, t, :].T, np.float32(0.0), np.float32(NEG))

    def onehot_tile(g, t):
        # [128 i, 512 q] bf16: one-hot at i = first dropped k (suffix drop)
        drop = ~mm[g, :, t, :]                  # [qr, kc]
        any_drop = drop.any(axis=1)
        k0 = np.argmax(drop, axis=1)
        b = np.zeros((P, QC), np.float32)
        b[k0[any_drop], np.nonzero(any_drop)[0]] = 1.0
        return b.astype(BF)

    in_maps = []
    for c in range(N_CORES):
        b, par = divmod(c, 2)
        chunks = [2 * s + par for s in range(N_SLOTS)]
        qT_core = _chunk_major(np.concatenate(
            [qTb[b][:, g * QC:(g + 1) * QC] for g in chunks], axis=1))
        if mode == 'stream':
            order = sorted(
                ((s, t) for s in range(N_SLOTS) for t in window[s]),
                key=lambda st: (st[1], st[0]))
            mp = np.stack([add_tile(chunks[s], t) for (s, t) in order]
                          ).astype(np.float32)
        elif nj:
            tiles = []
            for j in range(nj):
                s = next(s for s in range(N_SLOTS) if j < len(window[s]))
                g, t = chunks[s], window[s][j]
                tiles.append(onehot_tile(g, t) if mode == 'mm'
                             else add_tile(g, t).astype(np.float32))
            # [nj, 128, 512] -> [128, nj, 512]
            mp = np.ascontiguousarray(np.stack(tiles).transpose(1, 0, 2))
        else:
            mp = np.zeros((P, 1, QC), BF if mode == 'mm' else np.float32)
        in_maps.append({
            "qT": qT_core, "kT": kTb[b], "vT": vTb[b],
            "w": w2, "maskp": mp, "atri": atri,
        })
    return in_maps


def _gather_out(results):
    outp = np.empty((B, T, H), np.float32)
    for c in range(N_CORES):
        b, par = divmod(c, 2)
        oc = results[c]["out"]
        for s in range(N_SLOTS):
            g = 2 * s + par
            outp[b, g * QC:(g + 1) * QC, :] = oc[s * QC:(s + 1) * QC, :]
    return outp


def kernel(q, k, v, Wq, Wk, Wv, attn_mask):
    global LAST_RESULT
    q = np.asarray(q, dtype=np.float32)
    k = np.asarray(k, dtype=np.float32)
    v = np.asarray(v, dtype=np.float32)
    mask = np.asarray(attn_mask).astype(bool)

    extents, window, mode = _mask_schedule(mask)
    nc = _get_nc(extents, window, mode)
    in_maps = _make_in_maps(q, k, v, Wq, Wk, Wv, mask, extents, window, mode)

    res = run_bass_kernel_spmd(
        nc, in_maps, core_ids=list(range(N_CORES)),
        trace=bool(os.environ.get("KBENCH_TRACE")))
    LAST_RESULT = res
    return _gather_out(res.results)


# revision 27
# speedup vs baseline: 1.0318x; 1.0318x over previous
"""Distributed Trainium2 Bass kernel for a single attention head.

Problem (hardcoded): q,k,v [4, 4096, 1024] f32, Wq/Wk/Wv [1024, 64] f32,
attn_mask [4096, 4096] bool (True = keep).  out[b] = softmax(mask(q Wq (k Wk)^T) / 8) (v Wv).

Sharding: 8 cores; core c -> batch c//2, and query chunks {2s + c%2 : s in 0..3}
(512 rows each, interleaved so the causal-mask work per program slot is uniform
across cores -- all cores execute one SPMD program).

v7: bf16 operands (host casts).  PE is kept gapless so it stays at its
2.4 GHz p-state: causal mask applied on the PE (Atri^T @ B accumulated into
the score PSUM, B one-hot at first dropped k), chunk ch+1's projections
drained as filler between chunk ch's attention matmuls, PV pipelined one
group behind scores.  Attention work items (k-tile, slot) are packed two
per [128,1024] score PSUM in flat order -- pairs span t where only one slot
is live -- so one exp instruction always covers two items (halves the
activation-bound tail).  Host pre-arranges q/k/v/w/mask into the exact SBUF
layouts (chunk-major [128, dt, cols]) so every DMA is one contiguous run
per partition (cheap descriptor generation); input DMAs issue from the
otherwise-idle gpsimd queue.  Slot 3's first extents[0] k-tiles are
deferred to the tail so slots 0 and 3 share a PSUM bank, freeing a
dedicated projection-PSUM bank.
"""

import os
import sys

sys.path.insert(0, "/opt/trn_rl_repo")

import numpy as np
import ml_dtypes

import concourse.bass as bass
import concourse.mybir as mybir
import concourse.tile as tile
from concourse import bacc
from concourse.bass_utils import run_bass_kernel_spmd
from concourse.masks import make_identity

F32 = mybir.dt.float32
BF16 = mybir.dt.bfloat16
BF = ml_dtypes.bfloat16

N_CORES = 8
B, T, D, H = 4, 4096, 1024, 64
P = 128                      # partitions
QC = 512                     # query chunk width
N_CHUNKS = T // QC           # 8 global query chunks
N_SLOTS = N_CHUNKS // 2      # 4 chunks per core
KT = T // P                  # 32 k-tiles of 128 rows
D_TILES = D // P             # 8
XCW = 1024                   # x-chunk width for streaming projections
TQ = N_SLOTS * QC            # 2048 local query rows per core
KPC = XCW // P               # k-tiles per chunk
NQCH = TQ // XCW             # q chunks
NKCH = T // XCW              # k/v chunks (full extent)
NEG = -1.0e30                # additive mask value for dropped positions

LAST_RESULT = None           # test harness reads exec_time_ns from here
_CACHE = {}


def _mask_schedule(mask):
    """Derive the compile-time schedule from the actual mask.

    Returns (extents, window, mode):
      mode 'mm': window blocks are k-suffix drops, content slot-independent
                 per parity -> mask as a PE matmul (one-hot B tiles).
      mode 'add': consistent but not suffix -> resident additive DVE tiles.
      mode 'stream': general fallback, tiles streamed from HBM per (s,t).
    """
    m = mask.reshape(N_CHUNKS, QC, KT, P)
    blk_any = m.any(axis=(1, 3))   # [chunk, ktile]
    blk_all = m.all(axis=(1, 3))
    extents = []
    window = []
    for s in range(N_SLOTS):
        js = (2 * s, 2 * s + 1)
        ext = 1
        for j in js:
            nz = np.nonzero(blk_any[j])[0]
            if len(nz):
                ext = max(ext, int(nz[-1]) + 1)
        extents.append(ext)
        window.append(tuple(
            t for t in range(ext) if (~blk_all[js, t]).any()))
    nj = max((len(w) for w in window), default=0)
    consistent = nj <= 16
    suffix = True
    if consistent:
        for par in range(2):
            for j in range(nj):
                blocks = [
                    m[2 * s + par, :, window[s][j], :]
                    for s in range(N_SLOTS) if j < len(window[s])
                ]
                if any(not np.array_equal(blocks[0], bb) for bb in blocks[1:]):
                    consistent = False
                drop = ~blocks[0]            # [qr, kc]
                if not (drop[:, :-1] <= drop[:, 1:]).all():
                    suffix = False
    mode = ('mm' if consistent and suffix else
            'add' if consistent else 'stream')
    return tuple(extents), tuple(window), mode


def _build(extents, window, mode):
    nj = max((len(w) for w in window), default=0)
    n_mask = max(1, sum(len(w) for w in window))   # stream mode count
    mdt = BF16 if mode == 'mm' else F32
    nc = bacc.Bacc("TRN2", target_bir_lowering=False, debug=False,
                   num_devices=N_CORES)
    qT = nc.dram_tensor("qT", [NQCH, P, D_TILES, XCW], BF16,
                        kind="ExternalInput")
    kT = nc.dram_tensor("kT", [NKCH, P, D_TILES, XCW], BF16,
                        kind="ExternalInput")
    vT = nc.dram_tensor("vT", [NKCH, P, D_TILES, XCW], BF16,
                        kind="ExternalInput")
    w = nc.dram_tensor("w", [P, D_TILES, 3 * H], BF16, kind="ExternalInput")
    if mode == 'stream':
        maskp = nc.dram_tensor("maskp", [n_mask, P, QC], F32,
                               kind="ExternalInput")
    else:
        maskp = nc.dram_tensor("maskp", [P, max(1, nj), QC], mdt,
                               kind="ExternalInput")
    atri = nc.dram_tensor("atri", [P, P], BF16, kind="ExternalInput")
    out = nc.dram_tensor("out", [TQ, H], F32, kind="ExternalOutput")

    Exp = mybir.ActivationFunctionType.Exp
    kt_lim = max(extents)
    n_kv_chunks = (kt_lim * P + XCW - 1) // XCW
    jmap = {}
    for s in range(N_SLOTS):
        for j, t in enumerate(window[s]):
            jmap[(s, t)] = j
    stream_order = {}
    for t in range(kt_lim):
        for s in range(N_SLOTS):
            if t < extents[s] and (s, t) in jmap:
                stream_order[(s, t)] = len(stream_order)

    # slot-3 deferral: share one PSUM bank between slot 0 and slot 3 by
    # processing slot 3's first extents[0] k-tiles after everything else.
    DEF = N_SLOTS - 1
    E0 = extents[0]
    defer = extents[DEF] > E0
    def live_at(t, s):
        return t < extents[s] and not (defer and s == DEF and t < E0)

    items = []
    # chunk 0: slot-pair-major so early groups need only the first half of
    # q (slots 0,1) while q's second half is still loading
    for spair in range(2):
        for t in range(min(KPC, kt_lim)):
            for s in (2 * spair, 2 * spair + 1):
                if live_at(t, s):
                    items.append((t, s))
    for t in range(KPC, kt_lim):
        for s in range(N_SLOTS):
            if live_at(t, s):
                items.append((t, s))
    if defer:
        # zip the deferred slot-3 items into the final stretch (which is
        # mostly solo slot-3 k-tiles) so tail groups stay 2 items wide
        defi = [(t, DEF) for t in range(E0)]
        kzip = min(len(defi), len(items))
        tail = items[len(items) - kzip:]
        items = items[:len(items) - kzip]
        for a, b in zip(tail, defi):
            items.extend((a, b))
        items.extend(defi[kzip:])
    groups = [items[i:i + 2] for i in range(0, len(items), 2)]
    first_t = {}
    last_t = {}
    for t, s in items:
        first_t.setdefault(s, t)
        last_t[s] = t

    with tile.TileContext(nc) as tc:
        with (
            tc.tile_pool(name="const", bufs=1) as cpool,
            tc.tile_pool(name="qkh", bufs=1) as qkhpool,
            tc.tile_pool(name="vh", bufs=1) as vhpool,
            tc.tile_pool(name="oacc", bufs=1, space="PSUM") as opool,
        ):
            w_sb = cpool.tile([P, D_TILES, 3 * H], BF16)
            nc.sync.dma_start(out=w_sb[:], in_=w.ap())
            ident_b = cpool.tile([P, P], BF16)
            make_identity(nc, ident_b[:])
            ident_f = cpool.tile([H + 1, H + 1], F32)
            make_identity(nc, ident_f[:])
            atri_sb = None
            maskp_sb = None
            if mode == 'mm':
                atri_sb = cpool.tile([P, P], BF16)
                nc.sync.dma_start(out=atri_sb[:], in_=atri.ap())
            if mode != 'stream' and nj:
                maskp_sb = cpool.tile([P, nj, QC], mdt)
                # (its DMA is issued in the head sequence, after q0/k0)

            qhT = qkhpool.tile([H, TQ], BF16, tag="qhT")
            khT = qkhpool.tile([H, T], BF16, tag="khT")
            vh1 = vhpool.tile([P, KT, H + 1], BF16)

            oaccs = {}

            def oacc_alloc(s):
                tag = "oaccA" if (s in (0, DEF) and defer) else f"oacc{s}"
                oaccs[s] = opool.tile([H + 1, QC], F32, tag=tag,
                                      name=f"oacc{s}")
            for s in range(N_SLOTS):
                if not (defer and s == DEF):
                    oacc_alloc(s)

            with (
                tc.tile_pool(name="xs", bufs=4) as xpool,
                tc.tile_pool(name="ps", bufs=2, space="PSUM") as spool,
                tc.tile_pool(name="pps", bufs=1, space="PSUM") as pppool,
                tc.tile_pool(name="pt", bufs=6) as ppool,
                tc.tile_pool(name="vsb", bufs=2) as vsbpool,
                tc.tile_pool(name="mt", bufs=3) as mpool,
                tc.tile_pool(name="osb", bufs=2) as osbpool,
                tc.tile_pool(name="rec", bufs=2) as recpool,
                tc.tile_pool(name="ob", bufs=2) as obpool,
            ):
                proj_pool = pppool if defer else spool
                ppbig = None
                if defer:
                    # persistent [128, QC] PSUM bank: projection chains use
                    # alternating 64-partition halves (no WAR between
                    # consecutive chains); transposes/epilogue use [:, 0:65].
                    ppbig = pppool.tile([P, QC], F32, tag="pp", name="ppbig")
                chain_par = [0]

                def proj_psum():
                    if ppbig is None:
                        return proj_pool.tile([H, QC], F32, tag="pp")
                    r0 = chain_par[0] * H
                    chain_par[0] ^= 1
                    return ppbig[r0:r0 + H, :]

                def tp_psum(dtype):
                    if ppbig is None:
                        return proj_pool.tile([P, H + 1], dtype,
                                              tag="pp", name="tp")[:]
                    if dtype == F32:
                        return ppbig[:, 0:H + 1]
                    return ppbig[:].bitcast(dtype)[:, 0:H + 1]

                def proj_chain(dst, dst_col, src_sb, n, wlo, whi):
                    """8-matmul chain projecting src cols [n*QC,(n+1)*QC);
                    DVE-copies the psum into dst[0:H, dst_col:+QC]."""
                    pp = proj_psum()
                    for dt_ in range(D_TILES):
                        nc.tensor.matmul(
                            pp,
                            lhsT=w_sb[:, dt_, wlo:whi],
                            rhs=src_sb[:, dt_, n * QC:(n + 1) * QC],
                            start=(dt_ == 0), stop=(dt_ == D_TILES - 1))
                        if dt_ % 4 == 3:
                            yield
                    nc.vector.tensor_scalar_mul(
                        dst[0:H, dst_col:dst_col + QC], pp, 1.0)
                    yield

                def kv_gen(ch):
                    """chunk ch's k/v projection, yielded in filler units."""
                    base = ch * XCW
                    ktx, vtx = xts[ch]
                    for n in range(XCW // QC):
                        yield from proj_chain(khT, base + n * QC, ktx, n,
                                              H, 2 * H)
                    vsb = vsbpool.tile([H + 1, XCW], BF16, tag="vsb")
                    nc.vector.memset(vsb[H:H + 1, :], 1.0)
                    for n in range(XCW // QC):
                        yield from proj_chain(vsb, n * QC, vtx, n,
                                              2 * H, 3 * H)
                    for kt_ in range(KPC):
                        t_glob = ch * KPC + kt_
                        if t_glob >= kt_lim:
                            continue
                        tp = tp_psum(BF16)
                        nc.tensor.transpose(
                            tp, vsb[:, kt_ * P:(kt_ + 1) * P],
                            ident_b[0:H + 1, 0:H + 1])
                        nc.vector.tensor_scalar_mul(
                            vh1[:, t_glob, :], tp, 1.0)
                        if kt_ % 2 == 1:
                            yield

                def dma_k(ch):
                    ktx = xpool.tile([P, D_TILES, XCW], BF16, tag="x")
                    nc.sync.dma_start(out=ktx[:], in_=kT[ch])
                    return ktx

                def dma_v(ch):
                    vtx = xpool.tile([P, D_TILES, XCW], BF16, tag="x")
                    nc.sync.dma_start(out=vtx[:], in_=vT[ch])
                    return vtx

                def dma_chunk(ch):
                    xts[ch] = (dma_k(ch), dma_v(ch))

                def emit_group(group):
                    """scores (+mask) + one exp for <=2 (t, s) items."""
                    wt = spool.tile([P, 2 * QC], F32, tag="ps")
                    pt = ppool.tile([P, 2 * QC], BF16, tag="p")
                    for idx, (t, s) in enumerate(group):
                        ho = idx * QC
                        mm_mask = mode == 'mm' and (s, t) in jmap
                        nc.tensor.matmul(
                            wt[:, ho:ho + QC],
                            lhsT=khT[:, t * P:(t + 1) * P],
                            rhs=qhT[:, s * QC:(s + 1) * QC],
                            start=True, stop=not mm_mask)
                        if mm_mask:
                            nc.tensor.matmul(
                                wt[:, ho:ho + QC],
                                lhsT=atri_sb[:],
                                rhs=maskp_sb[:, jmap[(s, t)], :],
                                start=False, stop=True)
                        elif (s, t) in jmap:
                            if mode == 'add':
                                nc.vector.tensor_add(
                                    wt[:, ho:ho + QC], wt[:, ho:ho + QC],
                                    maskp_sb[:, jmap[(s, t)], :])
                            else:
                                m = mpool.tile([P, QC], F32, tag="m")
                                nc.sync.dma_start(
                                    out=m[:],
                                    in_=maskp[stream_order[(s, t)]])
                                nc.vector.tensor_add(
                                    wt[:, ho:ho + QC], wt[:, ho:ho + QC],
                                    m[:])
                    hi = len(group) * QC
                    nc.scalar.activation(
                        out=pt[:, 0:hi], in_=wt[:, 0:hi],
                        func=Exp, scale=0.125)
                    return [(t, s, pt, idx * QC)
                            for idx, (t, s) in enumerate(group)]

                def emit_pv(pv_items):
                    for (t, s, pt, ho) in pv_items:
                        if s not in oaccs:
                            oacc_alloc(s)
                        nc.tensor.matmul(
                            oaccs[s][:],
                            lhsT=vh1[:, t, :],
                            rhs=pt[:, ho:ho + QC],
                            start=(t == first_t[s]), stop=(t == last_t[s]))
                        if t == last_t[s]:
                            epilogue(s)

                def epilogue(s):
                    osb = osbpool.tile([H + 1, QC], F32, tag="osb")
                    nc.vector.tensor_scalar_mul(osb[:], oaccs[s][:], 1.0)
                    for jj in range(QC // P):
                        ot = tp_psum(F32)
                        nc.tensor.transpose(
                            ot, osb[:, jj * P:(jj + 1) * P], ident_f[:])
                        rec = recpool.tile([P, 1], F32, tag="rec")
                        nc.vector.reciprocal(rec[:], ot[:, H:H + 1])
                        ob = obpool.tile([P, H], F32, tag="ob")
                        nc.vector.tensor_scalar_mul(ob[:], ot[:, 0:H], rec[:])
                        r0 = s * QC + jj * P
                        nc.sync.dma_start(out=out[r0:r0 + P, :], in_=ob[:])

                xts = {}

                # head: issue DMAs in consumption order and project each
                # tensor as it arrives: q0, k0, masks, q1, v0
                q0x = xpool.tile([P, D_TILES, XCW], BF16, tag="x")
                nc.sync.dma_start(out=q0x[:], in_=qT[0])
                ktx0 = dma_k(0)
                if maskp_sb is not None:
                    nc.sync.dma_start(out=maskp_sb[:], in_=maskp.ap())
                q1x = None
                if NQCH > 1:
                    q1x = xpool.tile([P, D_TILES, XCW], BF16, tag="x")
                    nc.sync.dma_start(out=q1x[:], in_=qT[1])
                vtx0 = dma_v(0)
                xts[0] = (ktx0, vtx0)
                for n in range(XCW // QC):
                    for _ in proj_chain(qhT, n * QC, q0x, n, 0, H):
                        pass
                for n in range(XCW // QC):
                    for _ in proj_chain(khT, n * QC, ktx0, n, H, 2 * H):
                        pass
                if q1x is not None:
                    for n in range(XCW // QC):
                        for _ in proj_chain(qhT, XCW + n * QC, q1x, n, 0, H):
                            pass
                vsb0 = vsbpool.tile([H + 1, XCW], BF16, tag="vsb")
                nc.vector.memset(vsb0[H:H + 1, :], 1.0)
                for n in range(XCW // QC):
                    for _ in proj_chain(vsb0, n * QC, vtx0, n, 2 * H, 3 * H):
                        pass
                for kt_ in range(min(KPC, kt_lim)):
                    tp = tp_psum(BF16)
                    nc.tensor.transpose(
                        tp, vsb0[:, kt_ * P:(kt_ + 1) * P],
                        ident_b[0:H + 1, 0:H + 1])
                    nc.vector.tensor_scalar_mul(vh1[:, kt_, :], tp, 1.0)

                # flat group loop: drain chunk ch+1's projection as filler
                prev = None
                gen = None
                gen_ch = 0
                dma_hi = 0
                seen_ch = -1
                for group in groups:
                    ch = max(t for t, _ in group) // KPC
                    if ch > seen_ch:
                        seen_ch = ch
                        if gen is not None:
                            for _ in gen:
                                pass
                            gen = None
                        for c in (ch + 1, ch + 2):
                            if c < n_kv_chunks and c > dma_hi:
                                dma_chunk(c)
                                dma_hi = c
                        if ch + 1 < n_kv_chunks and ch + 1 > gen_ch:
                            gen = kv_gen(ch + 1)
                            gen_ch = ch + 1
                    cur = emit_group(group)
                    if gen is not None:
                        for _ in range(3):
                            if next(gen, 'DONE') == 'DONE':
                                gen = None
                                break
                    if prev is not None:
                        emit_pv(prev)
                    prev = cur
                if gen is not None:
                    for _ in gen:
                        pass
                if prev is not None:
                    emit_pv(prev)

    nc.compile()
    return nc


def _get_nc(extents, window, mode):
    key = (extents, window, mode)
    if key not in _CACHE:
        _CACHE[key] = _build(extents, window, mode)
    return _CACHE[key]


def _atri():
    i = np.arange(P)
    return (np.float32(NEG) * (i[:, None] <= i[None, :])).astype(BF)


def _chunk_major(xT):
    """[D, C] (d-major) -> [C//XCW, 128, 8, XCW] so each DMA chunk is one
    contiguous run per partition."""
    ncol = xT.shape[1]
    return np.ascontiguousarray(
        xT.reshape(D_TILES, P, ncol // XCW, XCW).transpose(2, 1, 0, 3))


def _make_in_maps(q, k, v, Wq, Wk, Wv, mask, extents, window, mode):
    nj = max((len(w) for w in window), default=0)
    wcat = np.concatenate(
        [np.asarray(Wq), np.asarray(Wk), np.asarray(Wv)], axis=1).astype(BF)
    w2 = np.ascontiguousarray(
        wcat.reshape(D_TILES, P, 3 * H).transpose(1, 0, 2))
    kTb = [_chunk_major(k[b].T.astype(BF)) for b in range(B)]
    vTb = [_chunk_major(v[b].T.astype(BF)) for b in range(B)]
    qTb = [np.ascontiguousarray(q[b].T.astype(BF)) for b in range(B)]
    mm = mask.reshape(N_CHUNKS, QC, KT, P)
    atri = _atri()

    def add_tile(g, t):
        # [128 k, 512 q] additive f32 tile for (chunk g, k-tile t)
        return np.where(mm[g, :, t, :].T, np.float32(0.0), np.float32(NEG))

    def onehot_tile(g, t):
        # [128 i, 512 q] bf16: one-hot at i = first dropped k (suffix drop)
        drop = ~mm[g, :, t, :]                  # [qr, kc]
        any_drop = drop.any(axis=1)
        k0 = np.argmax(drop, axis=1)
        b = np.zeros((P, QC), np.float32)
        b[k0[any_drop], np.nonzero(any_drop)[0]] = 1.0
        return b.astype(BF)

    in_maps = []
    for c in range(N_CORES):
        b, par = divmod(c, 2)
        chunks = [2 * s + par for s in range(N_SLOTS)]
        qT_core = _chunk_major(np.concatenate(
            [qTb[b][:, g * QC:(g + 1) * QC] for g in chunks], axis=1))
        if mode == 'stream':
            order = sorted(
                ((s, t) for s in range(N_SLOTS) for t in window[s]),
                key=lambda st: (st[1], st[0]))
            mp = np.stack([add_tile(chunks[s], t) for (s, t) in order]
                          ).astype(np.float32)
        elif nj:
            tiles = []
            for j in range(nj):
                s = next(s for s in range(N_SLOTS) if j < len(window[s]))
                g, t = chunks[s], window[s][j]
                tiles.append(onehot_tile(g, t) if mode == 'mm'
                             else add_tile(g, t).astype(np.float32))
            # [nj, 128, 512] -> [128, nj, 512]
            mp = np.ascontiguousarray(np.stack(tiles).transpose(1, 0, 2))
        else:
            mp = np.zeros((P, 1, QC), BF if mode == 'mm' else np.float32)
        in_maps.append({
            "qT": qT_core, "kT": kTb[b], "vT": vTb[b],
            "w": w2, "maskp": mp, "atri": atri,
        })
    return in_maps


def _gather_out(results):
    outp = np.empty((B, T, H), np.float32)
    for c in range(N_CORES):
        b, par = divmod(c, 2)
        oc = results[c]["out"]
        for s in range(N_SLOTS):
            g = 2 * s + par
            outp[b, g * QC:(g + 1) * QC, :] = oc[s * QC:(s + 1) * QC, :]
    return outp


def kernel(q, k, v, Wq, Wk, Wv, attn_mask):
    global LAST_RESULT
    q = np.asarray(q, dtype=np.float32)
    k = np.asarray(k, dtype=np.float32)
    v = np.asarray(v, dtype=np.float32)
    mask = np.asarray(attn_mask).astype(bool)

    extents, window, mode = _mask_schedule(mask)
    nc = _get_nc(extents, window, mode)
    in_maps = _make_in_maps(q, k, v, Wq, Wk, Wv, mask, extents, window, mode)

    res = run_bass_kernel_spmd(
        nc, in_maps, core_ids=list(range(N_CORES)),
        trace=bool(os.environ.get("KBENCH_TRACE")))
    LAST_RESULT = res
    return _gather_out(res.results)
